# revision 80
# baseline (speedup 1.0000x reference)
"""Causal self-attention on 8 trn2 NeuronCores.

Sharding: tensor-parallel over heads (2 heads/core) for QKV+attention; four
chunk-sized AllToAlls (one per pair of 512-query strips, fired as soon as
that pair's attention completes) reshard from head-split to row-split for
the output projection, so all but the last collective overlap compute and
the last one is data-bound, not queue-bound.

Per-core pipeline (engines):
 - x is DMAed batch-0 first; batch-0 V+Q+K projections run k-outer
   interleaved (8 matmuls per k-tile) so the PE rides the x DMA stream at
   full p-state; pair-1 Q/K and all batch-1 projections are emitted as
   chunks at strip boundaries, filling PE slack via the wait-queue bypass.
 - Attention is per-strip in "scores transposed" layout ([keys, queries]);
   both heads' scores live in one [128,1024] psum tile so one exp covers
   both; causal masking multiplies only the 128-col diagonal segment with a
   single [128,128] triangle mask, off the exp critical path.
 - Softmax denominators come from 64 replicated ones-columns appended to V,
   so the reciprocal is already partition-broadcast; no DRAM bounce.
 - Output projection is transposed (psum partitions = out-cols, moving dim
   = rows) so the bias folds into the DVE evac; per-chunk out-projections
   run inside collective/tail idle windows, with small collective-gated
   warm matmuls keeping the PE p-state up before the last ones.
"""

import numpy as np
import ml_dtypes

B, T, D, H, HD = 2, 2048, 1024, 16, 64
NCORES = 8
R = B * T              # 4096 global rows (b*T + t)
HPC = H // NCORES      # 2 heads per core
HDIM = HPC * HD        # 128 dims per core
NKT = D // 128         # 8 contraction tiles
NSTRIP = T // 512      # 4 query strips per batch
NQ = 4                 # quarters (strip-pairs); quarter q = (b=q//2, strips 2(q%2)..)

_BF16 = ml_dtypes.bfloat16
_cache = {}
# chunks of global strips (0..7); each strip contributes 64 rows/core
CHUNKS_HOST = [[0, 1], [2, 3], [4, 5], [6, 7]]
PHASES = []  # (label, first_inst_id) markers, for offline trace attribution


def _patch_tile_drain():
    """This walrus build rejects >1 sync wait on SP CTRL instructions; split
    the Tile tail-drain waits across single-wait nops."""
    import concourse.mybir as mybir
    import concourse.tile as tile_mod
    from concourse.vector_clock import ScopedClock

    if getattr(tile_mod.TileContext, "_drain_patched", False):
        return

    def _drain_and_barrier(self, tick_clock, wait_clock):
        nc = self.nc
        dummy = mybir.InstNoOp(
            name=nc.get_next_instruction_name(),
            engine=mybir.EngineType.SP,
            ins=[],
            outs=[],
        )
        wait_clock.add_sem_waits(dummy, ScopedClock({None: tick_clock.global_clock}))
        waits = list(dummy.sync_info.on_wait) if dummy.sync_info else []
        for i in range(len(waits)):
            w = nc.sync.nop(nofuse=True, hint="tail_drain_wait")
            w.ins.sync_info = mybir.SyncInfo(on_wait=waits[i : i + 1], on_update=[])
        nc.sync.drain()
        nc.all_engine_barrier()
        assert self.sems is not None
        popped = nc._tile_sem_poison_stack.pop()
        assert popped is self._sem_poison
        nc.clear_and_free_semaphores(list(self.sems.allocated().values()))
        nc.all_engine_barrier()

    tile_mod.TileContext._drain_and_barrier = _drain_and_barrier

    # Body instructions can also accumulate >2 waits (CTRL structs take 1,
    # other structs 2 on this walrus).  Before lowering, move excess waits
    # onto single-wait nops inserted just before the instruction on the same
    # engine stream.
    _orig_lower = tile_mod.TileContext._lower_ordered_insts

    def _lower_split_waits(self, ordered):
        nc = self.nc
        for bb_name, insts in ordered.items():
            new_insts = []
            for inst in insts:
                si = getattr(inst, "sync_info", None)
                waits = list(si.on_wait) if si is not None and si.on_wait else []
                limit = 1
                if len(waits) > limit and inst.engine is not None:
                    keep = waits[: limit - 1] if limit > 1 else []
                    spill = waits[len(keep) :][:-1]
                    keep = keep + [waits[-1]]
                    for w in spill:
                        nop = mybir.InstNoOp(
                            name=nc.get_next_instruction_name(),
                            engine=inst.engine,
                            ins=[],
                            outs=[],
                        )
                        nop.sync_info = mybir.SyncInfo(on_wait=[w], on_update=[])
                        nop.debug = inst.debug
                        new_insts.append(nop)
                    inst.sync_info = mybir.SyncInfo(
                        on_wait=keep, on_update=list(si.on_update or [])
                    )
                new_insts.append(inst)
            ordered[bb_name] = new_insts
        return _orig_lower(self, ordered)

    tile_mod.TileContext._lower_ordered_insts = _lower_split_waits
    tile_mod.TileContext._drain_patched = True


def _build():
    import concourse.bass as bass
    import concourse.mybir as mybir
    import concourse.tile as tile
    from concourse.tile import add_dep_helper
    from concourse.masks import make_identity

    _patch_tile_drain()
    f32 = mybir.dt.float32
    bf16 = mybir.dt.bfloat16

    nc = bass.Bass("TRN2", target_bir_lowering=False, debug=False, num_devices=NCORES)
    PHASES.clear()

    def mark(label):
        PHASES.append((label, nc.next_id()))

    # ---- DRAM I/O (per core) ----
    xT = nc.dram_tensor("xT", [D, R], bf16, kind="ExternalInput").ap()
    # weights pre-rearranged on host: [128, k, cols] flattened to [128, k*cols]
    wqT = nc.dram_tensor("wqT", [128, NKT * HDIM], bf16, kind="ExternalInput").ap()
    wkT = nc.dram_tensor("wkT", [128, NKT * HDIM], bf16, kind="ExternalInput").ap()
    wvT = nc.dram_tensor("wvT", [128, NKT * HDIM], bf16, kind="ExternalInput").ap()
    bq_s = nc.dram_tensor("bq_s", [HDIM, 1], f32, kind="ExternalInput").ap()
    bk_s = nc.dram_tensor("bk_s", [HDIM, 1], f32, kind="ExternalInput").ap()
    bv_s = nc.dram_tensor("bv_s", [HDIM, 1], f32, kind="ExternalInput").ap()
    woT = nc.dram_tensor("woT", [128, NKT * D], bf16, kind="ExternalInput").ap()
    # bias pre-broadcast for the transposed out-proj evac:
    # bo_bc[p, 128b+r] = bo[128b+p]
    bo_bc_d = nc.dram_tensor("bo_bc", [128, D], f32, kind="ExternalInput").ap()
    tri_d = nc.dram_tensor("tri", [128, 128], bf16, kind="ExternalInput").ap()
    # transposed output: outT[oc, 128q+r] = out[row 128q+r of quarter q, oc]
    outT = nc.dram_tensor("outT", [D, NQ * 128], f32, kind="ExternalOutput").ap()

    # collective bounce buffers, one per chunk of global strips.  Each strip
    # contributes 64 rows/core (512/8); shard j = rows [128j, 128(j+1)) =
    # head-dims x (this core's rows of the chunk, 64 per strip).  The last
    # chunk is a single strip so the final collective (which is fully
    # exposed) is small.
    CHUNKS = CHUNKS_HOST
    CHUNK_COLS = [64 * len(c) for c in CHUNKS]
    NCH = len(CHUNKS)
    STRIP_CHUNK = {gs: (q, i) for q, c in enumerate(CHUNKS) for i, gs in enumerate(c)}
    cc_in = [
        nc.dram_tensor(f"cc_in{q}", [NCORES * HDIM, w], bf16).ap()
        for q, w in enumerate(CHUNK_COLS)
    ]
    cc_out = [
        nc.dram_tensor(f"cc_out{q}", [NCORES * HDIM, w], bf16).ap()
        for q, w in enumerate(CHUNK_COLS)
    ]

    with tile.TileContext(nc) as tc:
        import contextlib

        with contextlib.ExitStack() as ctx:
            singles = ctx.enter_context(tc.tile_pool(name="singles", bufs=1))

            # ---- input DMAs ----
            # SP queue: wv then the x stream (b0 halves first) - the
            # projection-critical path.  Act queue: everything else (small,
            # plus wo late).
            w_sb = {}
            w_src = {"q": wqT, "k": wkT, "v": wvT}
            for name in ("q", "k", "v"):
                w_sb[name] = singles.tile(
                    [128, NKT, HDIM], bf16, tag=f"w{name}", name=f"w{name}"
                )
            nc.sync.dma_start(
                out=w_sb["v"], in_=wvT.rearrange("p (k c) -> p k c", k=NKT)
            )
            xt_sb = []  # [k][half] -> [128, 2048] tiles
            for k in range(NKT):
                t = singles.tile([128, R], bf16, tag=f"xt{k}", name=f"xt{k}")
                xt_sb.append(t)
            wo_sb = singles.tile([128, NKT, D], bf16, tag="wo")
            for bh in range(2):
                for k in range(NKT):
                    nc.sync.dma_start(
                        out=xt_sb[k][:, T * bh : T * (bh + 1)],
                        in_=xT[128 * k : 128 * (k + 1), T * bh : T * (bh + 1)],
                    )

            # big but non-urgent: issue after the x stream so it doesn't
            # steal DMA bandwidth from the projection-critical path
            nc.sync.dma_start(out=wo_sb, in_=woT.rearrange("p (k c) -> p k c", k=NKT))
            bias_sb = {}
            for name, src in (("v", bv_s), ("q", bq_s), ("k", bk_s)):
                bias_sb[name] = singles.tile(
                    [HDIM, 1], f32, tag=f"b{name}", name=f"b{name}"
                )
                nc.scalar.dma_start(out=bias_sb[name], in_=src)
            nc.scalar.dma_start(
                out=w_sb["q"], in_=wqT.rearrange("p (k c) -> p k c", k=NKT)
            )
            nc.scalar.dma_start(
                out=w_sb["k"], in_=wkT.rearrange("p (k c) -> p k c", k=NKT)
            )
            tri_sb = singles.tile([128, 128], bf16, tag="tri")
            nc.scalar.dma_start(out=tri_sb, in_=tri_d)
            bo_bc_sb = singles.tile([128, D], f32, tag="bobc")
            nc.scalar.dma_start(out=bo_bc_sb, in_=bo_bc_d)

            ident = singles.tile([128, 128], bf16, tag="ident")
            make_identity(nc, ident)
            # preload the Exp activation table off the critical path
            exp_warm = singles.tile([1, 128], bf16, tag="expwarm")
            nc.scalar.activation(
                out=exp_warm,
                in_=ident[0:1, :],
                func=mybir.ActivationFunctionType.Exp,
                scale=1.0,
            )

            # persistent activations
            qT_sb = singles.tile([128, R], bf16, tag="qT")   # rows 0-63 head A dims
            kT_sb = singles.tile([128, R], bf16, tag="kT")
            vT_sb = singles.tile([128, R], bf16, tag="vT")
            # v in [keys, dims] layout per key block kb:
            #   cols 0:64 = vA, 64:128 = ones, 128:192 = vB, 192:256 = ones
            # lhsT for head h = cols [128h, 128h+128): attnout at psum rows
            # 0-63, softmax sums replicated at rows 64-127.
            v_ab = singles.tile([128, R // 128, 256], bf16, tag="vab")
            nc.gpsimd.memset(
                v_ab.rearrange("p b (g c) -> p b g c", c=64)[:, :, 1::2, :], 1.0
            )
            af_sb = [
                singles.tile([128, NCORES, w], bf16, tag=f"af{q}", name=f"af{q}")
                for q, w in enumerate(CHUNK_COLS)
            ]

            scale = 1.0 / float(np.sqrt(HD))

            def proj_kouter(name, dest, bh, pool):
                """k-outer projection of batch-half bh into dest cols."""
                ps = [
                    pool.tile([128, 512], f32, tag="pp0", name=f"p0_{name}{bh}{n}")
                    for n in range(4)
                ]
                for k in range(NKT):
                    for n in range(4):
                        c0 = T * bh + 512 * n
                        nc.tensor.matmul(
                            ps[n],
                            w_sb[name][:, k, :],
                            xt_sb[k][:, c0 : c0 + 512],
                            start=(k == 0),
                            stop=(k == NKT - 1),
                        )
                for n in range(4):
                    c0 = T * bh + 512 * n
                    nc.vector.tensor_scalar_add(
                        dest[:, c0 : c0 + 512], ps[n], bias_sb[name]
                    )

            def proj_npair(name, dest, bh, npair, pool):
                """n-outer (2 psum tiles), k-inner projection chunk."""
                ps = []
                for i, n in enumerate((2 * npair, 2 * npair + 1)):
                    p = pool.tile([128, 512], f32, tag="ck", name=f"p1_{name}{bh}{n}")
                    ps.append((n, p))
                for k in range(NKT):
                    for n, p in ps:
                        c0 = T * bh + 512 * n
                        nc.tensor.matmul(
                            p,
                            w_sb[name][:, k, :],
                            xt_sb[k][:, c0 : c0 + 512],
                            start=(k == 0),
                            stop=(k == NKT - 1),
                        )
                for n, p in ps:
                    c0 = T * bh + 512 * n
                    nc.vector.tensor_scalar_add(dest[:, c0 : c0 + 512], p, bias_sb[name])

            def v_transpose(bh, lo, hi, pool, tag="vtp", copy_eng=None):
                """Transpose vT columns into v_ab [keys, dims] blocks,
                batched 4 blocks per psum tile + one strided DVE copy."""
                for kb0 in range(lo, hi, 4):
                    gkb0 = (T // 128) * bh + kb0
                    tp = pool.tile([128, 512], bf16, tag=tag)
                    for i in range(4):
                        kb = kb0 + i
                        nc.tensor.transpose(
                            tp[:, 128 * i : 128 * (i + 1)],
                            vT_sb[:, T * bh + 128 * kb : T * bh + 128 * (kb + 1)],
                            ident,
                        )
                    dst = v_ab[:, gkb0 : gkb0 + 4, :].rearrange(
                        "p b (g c) -> p b g c", c=64
                    )[:, :, ::2, :]
                    srcv = tp.rearrange("p (b g c) -> p b g c", b=4, g=2)
                    if copy_eng is nc.scalar:
                        nc.scalar.copy(dst, srcv)
                    else:
                        (copy_eng or nc.vector).tensor_copy(dst, srcv)

            mark("b0_proj")
            # ---- phase 1: batch-0 projections.  V + first halves of Q and K
            # run k-outer interleaved (8 matmuls per k-tile) so the PE stays
            # saturated (and p-state ramped) while riding the x DMA stream.
            with (
                tc.tile_pool(name="pp0", bufs=4, space="PSUM") as pp0,
                tc.tile_pool(name="qp0", bufs=2, space="PSUM") as qp0,
                tc.tile_pool(name="kp0", bufs=2, space="PSUM") as kp0,
            ):
                ps_v = [
                    pp0.tile([128, 512], f32, tag="pp0", name=f"p0_v{n}")
                    for n in range(4)
                ]
                ps_q = [
                    qp0.tile([128, 512], f32, tag="qp0", name=f"p0_q{n}")
                    for n in range(2)
                ]
                ps_k = [
                    kp0.tile([128, 512], f32, tag="kp0", name=f"p0_k{n}")
                    for n in range(2)
                ]
                for k in range(NKT):
                    for n in range(4):
                        nc.tensor.matmul(
                            ps_v[n],
                            w_sb["v"][:, k, :],
                            xt_sb[k][:, 512 * n : 512 * (n + 1)],
                            start=(k == 0),
                            stop=(k == NKT - 1),
                        )
                    for n in range(2):
                        nc.tensor.matmul(
                            ps_q[n],
                            w_sb["q"][:, k, :],
                            xt_sb[k][:, 512 * n : 512 * (n + 1)],
                            start=(k == 0),
                            stop=(k == NKT - 1),
                        )
                    for n in range(2):
                        nc.tensor.matmul(
                            ps_k[n],
                            w_sb["k"][:, k, :],
                            xt_sb[k][:, 512 * n : 512 * (n + 1)],
                            start=(k == 0),
                            stop=(k == NKT - 1),
                        )
                # evacs: the strip-0-critical K/Q first halves on DVE (the
                # first scores' wait only covers these), the rest on Act
                nc.vector.tensor_scalar_add(kT_sb[:, 0:512], ps_k[0], bias_sb["k"])
                nc.vector.tensor_scalar_add(qT_sb[:, 0:512], ps_q[0], bias_sb["q"])
                nc.scalar.add(kT_sb[:, 512:1024], ps_k[1], bias_sb["k"])
                nc.scalar.add(qT_sb[:, 512:1024], ps_q[1], bias_sb["q"])
                for n in range(4):
                    nc.scalar.add(
                        vT_sb[:, 512 * n : 512 * (n + 1)], ps_v[n], bias_sb["v"]
                    )
                v_transpose(0, 0, 4, pp0, tag="pp0")

            # ---- phase 2+3: attention (per strip) with batch-1 projections
            # and per-quarter collectives + output projection interleaved.
            cc_writes = {q: [] for q in range(NCH)}
            cc_insts = {}
            stage = {"i": 0}

            with (
                tc.tile_pool(name="sc_ps", bufs=2, space="PSUM") as scp,
                tc.tile_pool(name="av_ps", bufs=2, space="PSUM") as avp,
                tc.tile_pool(name="ck_ps", bufs=2, space="PSUM") as ckp,
                tc.tile_pool(name="p_sb", bufs=6) as ppool,
                tc.tile_pool(name="att_sb", bufs=6) as apool,
                tc.tile_pool(name="rec_sb", bufs=6) as rpool,
                tc.tile_pool(name="out_sb", bufs=4) as opool,
            ):

                def emit_strip(b, s):
                    """Attention for strip s of batch b (queries 512s..)."""
                    mark(f"strip{b}{s}")
                    qc0 = T * b + 512 * s
                    psV = {}
                    for hi in range(HPC):
                        psV[hi] = avp.tile(
                            [128, 512], f32, tag="av", name=f"psV_{b}{s}{hi}"
                        )
                    nkb = 4 * (s + 1)

                    def emit_av(kb, p, off):
                        gkb = (T // 128) * b + kb
                        for hi in range(HPC):
                            nc.tensor.matmul(
                                psV[hi][:, off:512],
                                v_ab[:, gkb, 128 * hi : 128 * (hi + 1)],
                                p[:, 512 * hi + off : 512 * (hi + 1)],
                                start=(kb == 0),
                                stop=(kb == nkb - 1),
                            )

                    pend = []
                    for kb in range(nkb):
                        krange = slice(T * b + 128 * kb, T * b + 128 * (kb + 1))
                        m = kb - 4 * s
                        off = 128 * m if m >= 0 else 0
                        psS = scp.tile([128, 1024], f32, tag="sc")
                        p = ppool.tile([128, 1024], bf16, tag="p")
                        for hi in range(HPC):
                            rows = slice(64 * hi, 64 * (hi + 1))
                            nc.tensor.matmul(
                                psS[:, 512 * hi + off : 512 * (hi + 1)],
                                kT_sb[rows, krange],
                                qT_sb[rows, qc0 + off : qc0 + 512],
                                start=True,
                                stop=True,
                                tile_position=(64 * hi, 0),
                            )
                        # one exp covers both heads; for diag blocks the
                        # [512:512+off] gap holds stale-but-finite psum that
                        # no AV ever reads
                        nc.scalar.activation(
                            out=p[:, off : 1024],
                            in_=psS[:, off : 1024],
                            func=mybir.ActivationFunctionType.Exp,
                            scale=scale,
                        )
                        if m >= 0:
                            # mask the 128-col diagonal segment of each head
                            seg = p.rearrange("p (h c) -> p h c", h=2)[
                                :, :, off : off + 128
                            ]
                            tri_b = bass.AP(
                                tensor=tri_sb.tensor,
                                offset=tri_sb.offset,
                                ap=[list(tri_sb.ap[0]), [0, 2], [1, 128]],
                            )
                            nc.vector.tensor_mul(seg, seg, tri_b)
                        pend.append((kb, p, off))
                        if len(pend) > 2:
                            emit_av(*pend.pop(0))
                    for a in pend:
                        emit_av(*a)
                    # normalize + extract: rec = 1/sums (rows 64-127 hold the
                    # replicated sums), att = attnout * rec
                    gs = 4 * b + s  # global strip 0..7
                    q, idx = STRIP_CHUNK[gs]
                    W = CHUNK_COLS[q]
                    recs = []
                    for hi in range(HPC):
                        rec = rpool.tile([64, 512], bf16, tag="rec")
                        with nc.allow_low_precision("bf16 softmax recip"):
                            nc.vector.reciprocal(rec, psV[hi][64:128, :])
                        recs.append(rec)
                    for hi in range(HPC):
                        dma_eng = nc.sync if hi == 0 else nc.scalar
                        att = apool.tile([64, 512], bf16, tag="att")
                        nc.vector.tensor_mul(att, psV[hi][0:64, :], recs[hi])
                        dst = bass.AP(
                            tensor=cc_in[q].tensor,
                            offset=cc_in[q].offset + 64 * hi * W + 64 * idx,
                            ap=[[W, 64], [128 * W, 8], [1, 64]],
                        )
                        wr = dma_eng.dma_start(
                            out=dst, in_=att.rearrange("p (j c) -> p j c", j=8)
                        )
                        cc_writes[q].append(wr)

                def emit_quarter_cc(q):
                    mark(f"cc{q}")
                    cc = nc.gpsimd.collective_compute(
                        "AllToAll",
                        mybir.AluOpType.bypass,
                        ins=[cc_in[q]],
                        outs=[cc_out[q]],
                        replica_groups=[list(range(NCORES))],
                    )
                    for wr in cc_writes[q]:
                        add_dep_helper(cc.ins, wr.ins, sync=True, reason="cc in ready")
                    cc_insts[q] = cc

                def emit_outproj(q):
                    """Transposed out-proj: psum [128 outcols-in-block, rows],
                    rows are the moving dim; bias folds into the DVE evac.
                    Chunks wider than 128 rows are split into row ranges."""
                    mark(f"outproj{q}")
                    W = CHUNK_COLS[q]
                    col0 = sum(CHUNK_COLS[:q])
                    # split the gather in two so the first i-half's matmuls
                    # can start while the second half is still in flight
                    for ilo in (0, 4):
                        rd = nc.scalar.dma_start(
                            out=af_sb[q][:, ilo : ilo + 4, :],
                            in_=cc_out[q].rearrange("(i p) c -> p i c", p=128)[
                                :, ilo : ilo + 4, :
                            ],
                        )
                        add_dep_helper(
                            rd.ins, cc_insts[q].ins, sync=True, reason="cc out ready"
                        )
                    step = 64 if q == NCH - 1 else 128
                    for r0 in range(0, W, step):
                        w = min(step, W - r0)
                        o_sb = opool.tile([128, NKT * w], f32, tag="osb")
                        ps = scp.tile([128, NKT * w], f32, tag="sc", name=f"op{q}_{r0}")
                        for ocb in range(D // 128):
                            for i in range(NCORES):
                                nc.tensor.matmul(
                                    ps[:, w * ocb : w * (ocb + 1)],
                                    wo_sb[:, i, 128 * ocb : 128 * (ocb + 1)],
                                    af_sb[q][:, i, r0 : r0 + w],
                                    start=(i == 0),
                                    stop=(i == NCORES - 1),
                                )
                        for blo in (0, 4):
                            bias = bo_bc_sb.rearrange("p (b r) -> p b r", r=128)[
                                :, blo : blo + 4, 0:w
                            ]
                            nc.vector.tensor_add(
                                o_sb.rearrange("p (b r) -> p b r", r=w)[:, blo : blo + 4],
                                ps.rearrange("p (b r) -> p b r", r=w)[:, blo : blo + 4],
                                bias,
                            )
                            # outT[128b+p, col0+r0+r] <- o_sb[p, w*b+r]
                            dst = bass.AP(
                                tensor=outT.tensor,
                                offset=outT.offset + blo * 128 * NQ * 128 + col0 + r0,
                                ap=[[NQ * 128, 128], [128 * NQ * 128, 4], [1, w]],
                            )
                            nc.sync.dma_start(
                                out=dst,
                                in_=o_sb.rearrange("p (b r) -> p b r", r=w)[:, blo : blo + 4],
                            )

                # deferred projection / transpose chunks, emitted at strip
                # boundaries (they fill PE slack via the wait-queue bypass)
                def chunk(label, *work):
                    mark(label)
                    for fn, args in work:
                        fn(*args)

                # ---- batch 0 ----
                emit_strip(0, 0)
                chunk(
                    "b0_q1",
                    (v_transpose, (0, 4, 8, ckp, "ck")),
                    (proj_npair, ("q", qT_sb, 0, 1, ckp)),
                )
                emit_strip(0, 1)
                chunk(
                    "b0_k1",
                    (v_transpose, (0, 8, 16, ckp, "ck")),
                    (proj_npair, ("k", kT_sb, 0, 1, ckp)),
                )
                emit_quarter_cc(0)
                chunk(
                    "b1_v",
                    (proj_npair, ("v", vT_sb, 1, 0, ckp)),
                    (proj_npair, ("v", vT_sb, 1, 1, ckp)),
                )
                emit_strip(0, 2)
                chunk(
                    "b1_qvt",
                    (proj_npair, ("q", qT_sb, 1, 0, ckp)),
                    (proj_npair, ("q", qT_sb, 1, 1, ckp)),
                    (v_transpose, (1, 0, 16, ckp, "ck")),
                )
                emit_strip(0, 3)
                emit_quarter_cc(1)
                chunk(
                    "b1_k",
                    (proj_npair, ("k", kT_sb, 1, 0, ckp)),
                    (proj_npair, ("k", kT_sb, 1, 1, ckp)),
                )
                # ---- batch 1 ----
                emit_outproj(0)
                emit_strip(1, 0)
                emit_strip(1, 1)
                emit_quarter_cc(2)
                emit_strip(1, 2)
                emit_strip(1, 3)
                emit_quarter_cc(3)
                emit_outproj(1)
                for ccq in (2, 3):
                    warm = scp.tile([128, 512], f32, tag="sc", name=f"warm{ccq}")
                    for i in range(4):
                        mm = nc.tensor.matmul(
                            warm, ident, kT_sb[:, 0:512], start=True, stop=True
                        )
                        if i == 0:
                            add_dep_helper(
                                mm.ins, cc_insts[ccq].ins, sync=True,
                                reason="pe warm",
                            )
                    emit_outproj(ccq)

    return nc


def _host_prep(x, Wq, bq, Wk, bk, Wv, bv, Wo, bo):
    """Build the 8 per-core input maps."""
    x = np.asarray(x, np.float32)
    xT = np.ascontiguousarray(x.reshape(R, D).T).astype(_BF16)

    def w_rearrange(w):  # [D, C] -> [128, NKT*C] with k-tiles interleaved
        D_, C = w.shape
        return np.ascontiguousarray(
            w.reshape(NKT, 128, C).transpose(1, 0, 2).reshape(128, NKT * C)
        )

    woT = w_rearrange(np.ascontiguousarray(np.asarray(Wo, np.float32).T)).astype(_BF16)
    # bo_bc[p, 128b+r] = bo[128b+p]
    bo_f = np.asarray(bo, np.float32)
    bo_bc = np.ascontiguousarray(
        np.broadcast_to(bo_f.reshape(NKT, 128).T[:, :, None], (128, NKT, 128)).reshape(
            128, D
        )
    )

    r = np.arange(128)[:, None]
    c = np.arange(128)[None, :]
    tri = (r <= c).astype(np.float32).astype(_BF16)

    in_maps = []
    for core in range(NCORES):
        hs = slice(HDIM * core, HDIM * (core + 1))
        in_maps.append(
            {
                "xT": xT,
                "wqT": w_rearrange(np.asarray(Wq, np.float32)[hs, :].T).astype(_BF16),
                "wkT": w_rearrange(np.asarray(Wk, np.float32)[hs, :].T).astype(_BF16),
                "wvT": w_rearrange(np.asarray(Wv, np.float32)[hs, :].T).astype(_BF16),
                "bq_s": np.asarray(bq, np.float32)[hs].reshape(HDIM, 1).copy(),
                "bk_s": np.asarray(bk, np.float32)[hs].reshape(HDIM, 1).copy(),
                "bv_s": np.asarray(bv, np.float32)[hs].reshape(HDIM, 1).copy(),
                "woT": woT,
                "bo_bc": bo_bc,
                "tri": tri,
            }
        )
    return in_maps


def _run(in_maps, trace=False):
    from concourse import bass_utils

    if "nc" not in _cache:
        _cache["nc"] = _build()
    nc = _cache["nc"]
    try:
        res = bass_utils.run_bass_kernel_spmd(
            nc, in_maps, core_ids=list(range(NCORES)), trace=False
        )
    except Exception:
        # transient device faults (NRT_EXEC_UNIT_UNRECOVERABLE) clear on retry
        res = bass_utils.run_bass_kernel_spmd(
            nc, in_maps, core_ids=list(range(NCORES)), trace=False
        )
    return res


def kernel(x, Wq, bq, Wk, bk, Wv, bv, Wo, bo, _trace=False, _want_results=False):
    in_maps = _host_prep(x, Wq, bq, Wk, bk, Wv, bv, Wo, bo)
    res = None
    for attempt in range(3):
        try:
            res = _run(in_maps, trace=_trace)
            full = np.zeros((B, T, D), np.float32)
            for j in range(NCORES):
                part = np.ascontiguousarray(res.results[j]["outT"].T)  # [512, D]
                col0 = 0
                for chunk in CHUNKS_HOST:
                    for gs in chunk:
                        b, r0 = gs // 4, 512 * (gs % 4) + 64 * j
                        full[b, r0 : r0 + 64, :] = part[col0 : col0 + 64, :]
                        col0 += 64
            break
        except Exception:
            # transient device faults (NRT_EXEC_UNIT_UNRECOVERABLE) can
            # surface when materializing results; retry the whole run
            if attempt == 2:
                raise
    if _want_results:
        return full, res
    return full
